# revision 1
# baseline (speedup 1.0000x reference)
"""MinGRU Trainium2 kernel.

Problem: x (8, 4096, 1024) fp32; Wz, Wh (1024, 1024); bz, bh (1024,).
    k = x @ Wz.T + bz ; z = sigmoid(k)
    p = x @ Wh.T + bh ; g = where(p >= 0, p + 0.5, sigmoid(p))
    h_t = (1 - z_t) * h_{t-1} + z_t * g_t   (h_0 = 0.5)
The reference computes this recurrence with a log-space parallel scan; here it
is computed directly in linear space (mathematically identical), using the DVE
TensorTensorScanArith instruction along the free axis.

Sharding: data-parallel over batch, one batch element per NeuronCore (8 cores).

Per-core layout: everything lives transposed, H on partitions, S on the free
axis.  k/p tiles (128, 512) come out of PSUM from 8-step K-accumulated
float32r matmuls (fp32 bits, full-rate PE streaming); bias adds are fused into
the ScalarE activations (per-partition bias); g = min(sigmoid(p+bh), 0.5) +
relu(p+bh) (identical to the where() branch).  b = z*g runs on the otherwise
idle GpSimd engine to keep the DVE below the PE roofline.
"""

import os
import sys

import numpy as np

for _p in ("/opt/trn_rl_repo", "/root/.axon_site/_ro/trn_rl_repo"):
    if os.path.isdir(_p) and _p not in sys.path:
        sys.path.insert(0, _p)

import concourse.bass as bass  # noqa: E402
import concourse.mybir as mybir  # noqa: E402
import concourse.tile as tile  # noqa: E402
from concourse import bacc  # noqa: E402
from concourse.bass_utils import run_bass_kernel_spmd  # noqa: E402

F32 = mybir.dt.float32
F32R = mybir.dt.float32r  # fp32 bits, full-rate PE streaming mode
N_CORES = 8
B, S, D, H = 8, 4096, 1024, 1024
TS = 512  # sequence strip width (= fp32 matmul max moving free dim)
NK = D // 128
NM = H // 128

_cache: dict = {}


def build_nc(seq_len: int = S, n_cores: int = N_CORES):
    """Build and compile the per-core Bass module (SPMD, identical program)."""
    nt = seq_len // TS
    nc = bacc.Bacc(
        "TRN2", target_bir_lowering=False, debug=False, num_devices=n_cores
    )

    xT_d = nc.dram_tensor("xT", [D, seq_len], F32R, kind="ExternalInput")
    wzT_d = nc.dram_tensor("wzT", [D, H], F32R, kind="ExternalInput")
    whT_d = nc.dram_tensor("whT", [D, H], F32R, kind="ExternalInput")
    bz_d = nc.dram_tensor("bz", [H], F32, kind="ExternalInput")
    bh_d = nc.dram_tensor("bh", [H], F32, kind="ExternalInput")
    hT_d = nc.dram_tensor("hT", [H, seq_len], F32, kind="ExternalOutput")

    AF = mybir.ActivationFunctionType
    OP = mybir.AluOpType

    with tile.TileContext(nc) as tc:
        with (
            tc.tile_pool(name="singles", bufs=1) as singles,
            tc.tile_pool(name="xs", bufs=3) as xpool,
            tc.tile_pool(name="work", bufs=3) as work,
            tc.tile_pool(name="hbuf", bufs=2) as hpool,
            tc.tile_pool(name="psum", bufs=4, space="PSUM") as psum,
        ):
            # PE warm-up: the HAM clock gate holds the PE at 1.2 GHz until it
            # has been busy ~3.4 us.  The PE sits idle for ~8 us anyway while
            # the first DMAs land, so burn that time on dummy matmuls over a
            # zeroed tile — the first real matmuls then run at 2.4 GHz.
            warm = singles.tile([128, TS], F32, tag="warm")
            nc.gpsimd.memset(warm[:], 0.0)
            wps = psum.tile([128, TS], F32, tag="kp")
            for i in range(12):
                nc.tensor.matmul(
                    wps[:], lhsT=warm[:, :128].bitcast(F32R),
                    rhs=warm[:].bitcast(F32R),
                    start=(i == 0), stop=(i == 11),
                )
            # First strip of x before the weights: the first matmuls need
            # xs(s=0) + the m<4 half of the weights, so order the initial DMAs
            # to unblock the PE as early as possible.
            # Biases first: they are tiny but gate every activation (and the
            # activations drain PSUM for the PE), so they must not queue
            # behind 8 MB of weights on the serialized DMA stream.
            bz_sb = singles.tile([128, NM], F32, tag="bz")
            nc.sync.dma_start(out=bz_sb, in_=bz_d.ap().rearrange("(m p) -> p m", p=128))
            bh_sb = singles.tile([128, NM], F32, tag="bh")
            nc.sync.dma_start(out=bh_sb, in_=bh_d.ap().rearrange("(m p) -> p m", p=128))
            # Initial DMA order matters: matmul (s=0, m=0) accumulates k-tiles
            # in order, so interleave per-k [xs0, wz-half0, wh-half0] loads —
            # the first matmul unblocks after ~3 transfers instead of 6 MB.
            xs0 = [None] * NK
            wz_sb = [[None, None] for _ in range(NK)]
            wh_sb = [[None, None] for _ in range(NK)]
            for k in range(NK):
                xt = xpool.tile([128, TS], F32R, tag=f"xs{k}")
                nc.sync.dma_start(out=xt, in_=xT_d.ap()[k * 128:(k + 1) * 128, 0:TS])
                xs0[k] = xt
                wz = singles.tile([128, H // 2], F32R, tag=f"wz{k}_0")
                nc.sync.dma_start(
                    out=wz, in_=wzT_d.ap()[k * 128:(k + 1) * 128, 0:H // 2]
                )
                wz_sb[k][0] = wz
                wh = singles.tile([128, H // 2], F32R, tag=f"wh{k}_0")
                nc.sync.dma_start(
                    out=wh, in_=whT_d.ap()[k * 128:(k + 1) * 128, 0:H // 2]
                )
                wh_sb[k][0] = wh
            for k in range(NK):
                wz = singles.tile([128, H // 2], F32R, tag=f"wz{k}_1")
                nc.sync.dma_start(
                    out=wz, in_=wzT_d.ap()[k * 128:(k + 1) * 128, H // 2:H]
                )
                wz_sb[k][1] = wz
                wh = singles.tile([128, H // 2], F32R, tag=f"wh{k}_1")
                nc.sync.dma_start(
                    out=wh, in_=whT_d.ap()[k * 128:(k + 1) * 128, H // 2:H]
                )
                wh_sb[k][1] = wh
            # The last 512-wide strip is split in two 256-wide strips (with b
            # on the DVE): the end-of-kernel pipeline drain runs on half-width
            # tiles, halving the post-matmul tail.
            strips = [(s * TS, TS) for s in range(nt - 1)]
            strips += [((nt - 1) * TS, TS // 2), ((nt - 1) * TS + TS // 2, TS // 2)]
            h_prev: list = [None] * NM

            def post_gemm(m, kp, pp, tw, ts_sl, last):
                """Gate math + scan + store for one (strip, m) unit."""
                z = work.tile([128, TS], F32, tag="z")
                nc.scalar.activation(
                    out=z[:, :tw], in_=kp[:, :tw], func=AF.Sigmoid,
                    bias=bz_sb[:, m:m + 1],
                )
                sp = work.tile([128, TS], F32, tag="sp")
                nc.scalar.activation(
                    out=sp[:, :tw], in_=pp[:, :tw], func=AF.Sigmoid,
                    bias=bh_sb[:, m:m + 1],
                )
                rp = work.tile([128, TS], F32, tag="rp")
                nc.scalar.activation(
                    out=rp[:, :tw], in_=pp[:, :tw], func=AF.Relu,
                    bias=bh_sb[:, m:m + 1],
                )
                # a = 1 - z
                a = work.tile([128, TS], F32, tag="a")
                nc.vector.tensor_scalar(
                    out=a[:, :tw], in0=z[:, :tw], scalar1=-1.0, scalar2=1.0,
                    op0=OP.mult, op1=OP.add,
                )
                # g = min(sigmoid(p+bh), 0.5) + relu(p+bh)
                g = work.tile([128, TS], F32, tag="g")
                nc.vector.scalar_tensor_tensor(
                    out=g[:, :tw], in0=sp[:, :tw], scalar=0.5, in1=rp[:, :tw],
                    op0=OP.min, op1=OP.add,
                )
                # b = z * g  (GpSimd: keeps DVE under the PE roofline;
                # DVE on the final strips, where drain latency matters)
                b = work.tile([128, TS], F32, tag="b")
                beng = nc.vector if last else nc.gpsimd
                beng.tensor_tensor(
                    out=b[:, :tw], in0=z[:, :tw], in1=g[:, :tw], op=OP.mult
                )
                # h_t = a_t * h_{t-1} + b_t along the free axis
                h = hpool.tile([128, TS], F32, tag=f"h{m}")
                if h_prev[m] is None:
                    init = 0.5
                else:
                    pt, pw = h_prev[m]
                    init = pt[:, pw - 1:pw]
                nc.vector.tensor_tensor_scan(
                    out=h[:, :tw], data0=a[:, :tw], data1=b[:, :tw],
                    initial=init, op0=OP.mult, op1=OP.add,
                )
                h_prev[m] = (h, tw)
                nc.sync.dma_start(out=hT_d.ap()[m * 128:(m + 1) * 128, ts_sl],
                                  in_=h[:, :tw])

            for s, (ts0, tw) in enumerate(strips):
                ts_sl = slice(ts0, ts0 + tw)
                last = ts0 + tw == seq_len
                if s == 0:
                    xs = xs0
                else:
                    xs = []
                    for k in range(NK):
                        xt = xpool.tile([128, TS], F32R, tag=f"xs{k}")
                        nc.sync.dma_start(
                            out=xt[:, :tw], in_=xT_d.ap()[k * 128:(k + 1) * 128, ts_sl]
                        )
                        xs.append(xt)
                for m in range(NM):
                    j, mj = divmod(m, NM // 2)
                    m_sl = slice(mj * 128, (mj + 1) * 128)
                    kp = psum.tile([128, TS], F32, tag="kp")
                    pp = psum.tile([128, TS], F32, tag="pp")
                    for k in range(NK):
                        nc.tensor.matmul(
                            kp[:, :tw],
                            lhsT=wz_sb[k][j][:, m_sl],
                            rhs=xs[k][:, :tw],
                            start=(k == 0),
                            stop=(k == NK - 1),
                        )
                    for k in range(NK):
                        nc.tensor.matmul(
                            pp[:, :tw],
                            lhsT=wh_sb[k][j][:, m_sl],
                            rhs=xs[k][:, :tw],
                            start=(k == 0),
                            stop=(k == NK - 1),
                        )
                    post_gemm(m, kp, pp, tw, ts_sl, last)

    nc.compile()
    return nc


def kernel(x, Wz, bz, Wh, bh):
    x = np.ascontiguousarray(x, dtype=np.float32)
    key = "nc"
    if key not in _cache:
        _cache[key] = build_nc()
    nc = _cache[key]

    wzT = np.ascontiguousarray(Wz.T.astype(np.float32))
    whT = np.ascontiguousarray(Wh.T.astype(np.float32))
    bz = np.ascontiguousarray(bz, dtype=np.float32)
    bh = np.ascontiguousarray(bh, dtype=np.float32)
    in_maps = [
        {
            "xT": np.ascontiguousarray(x[b].T),
            "wzT": wzT,
            "whT": whT,
            "bz": bz,
            "bh": bh,
        }
        for b in range(N_CORES)
    ]
    res = run_bass_kernel_spmd(nc, in_maps, list(range(N_CORES)))
    out = np.empty((B, S, H), np.float32)
    for b in range(N_CORES):
        out[b] = res.results[b]["hT"].T
    return out



# revision 2
# speedup vs baseline: 1.0971x; 1.0971x over previous
"""MinGRU Trainium2 kernel.

Problem: x (8, 4096, 1024) fp32; Wz, Wh (1024, 1024); bz, bh (1024,).
    k = x @ Wz.T + bz ; z = sigmoid(k)
    p = x @ Wh.T + bh ; g = where(p >= 0, p + 0.5, sigmoid(p))
    h_t = (1 - z_t) * h_{t-1} + z_t * g_t   (h_0 = 0.5)
The reference computes this recurrence with a log-space parallel scan; here it
is computed directly in linear space (mathematically identical), using the DVE
TensorTensorScanArith instruction along the free axis.

Sharding: data-parallel over batch, one batch element per NeuronCore (8 cores).

Per-core layout: everything lives transposed, H on partitions, S on the free
axis.  The two GEMMs run in fp8 e4m3 with perf_mode=DoubleRow (2 weights per
PE cell, K=256 per matmul -> half the matmul count of full-rate fp32).  Inputs
are quantized host-side with power-of-two scales (x*16, W*1024); the exact
descale 2^-14 is folded into the ScalarE activation `scale` argument, so the
only numerical deviation from the fp32 kernel is the fp8 rounding of x and W
(measured end-to-end rel-err ~1.3e-2 against the fp32 reference, within the
2e-2 budget).  k/p tiles (128, 512) come out of PSUM from 4-step K-accumulated
DoubleRow matmuls; bias adds are fused into the ScalarE activations
(per-partition bias); g = min(sigmoid(p+bh), 0.5) + relu(p+bh) (identical to
the where() branch).  b = z*g runs on the otherwise idle GpSimd engine to keep
the DVE below the PE roofline.
"""

import os
import sys

import numpy as np

for _p in ("/opt/trn_rl_repo", "/root/.axon_site/_ro/trn_rl_repo"):
    if os.path.isdir(_p) and _p not in sys.path:
        sys.path.insert(0, _p)

import ml_dtypes  # noqa: E402

import concourse.bass as bass  # noqa: E402
import concourse.mybir as mybir  # noqa: E402
import concourse.tile as tile  # noqa: E402
from concourse import bacc  # noqa: E402
from concourse.bass_utils import run_bass_kernel_spmd  # noqa: E402

F32 = mybir.dt.float32
F32R = mybir.dt.float32r
F8 = mybir.dt.float8e4  # TRN e4m3 (bias 8, max +-240) == ml_dtypes.float8_e4m3
NP_F8 = ml_dtypes.float8_e4m3
N_CORES = 8
B, S, D, H = 8, 4096, 1024, 1024
TS = 512  # sequence strip width (= one PSUM bank of fp32)
NK = D // 128  # 8 k-tiles of 128
NKP = NK // 2  # 4 DoubleRow k-pairs
NM = H // 128

# power-of-two quantization scales; descale folded into the activations
SX = 16.0
SW = 1024.0
DESCALE = 1.0 / (SX * SW)

_cache: dict = {}


def build_nc(seq_len: int = S, n_cores: int = N_CORES):
    """Build and compile the per-core Bass module (SPMD, identical program)."""
    nt = seq_len // TS
    nc = bacc.Bacc(
        "TRN2", target_bir_lowering=False, debug=False, num_devices=n_cores
    )

    # x packed host-side as [p, strip, ktile, t] so one DMA fetches a strip
    xp_d = nc.dram_tensor("xp8", [128, nt, NK, TS], F8, kind="ExternalInput")
    # weights packed as [p, ktile, m] (wz8[p, kt, m] = Wz[m, kt*128+p] * SW)
    wz_d = nc.dram_tensor("wz8", [128, NK, H], F8, kind="ExternalInput")
    wh_d = nc.dram_tensor("wh8", [128, NK, H], F8, kind="ExternalInput")
    bz_d = nc.dram_tensor("bz", [H], F32, kind="ExternalInput")
    bh_d = nc.dram_tensor("bh", [H], F32, kind="ExternalInput")
    hT_d = nc.dram_tensor("hT", [H, seq_len], F32, kind="ExternalOutput")

    AF = mybir.ActivationFunctionType
    OP = mybir.AluOpType
    DR = mybir.MatmulPerfMode.DoubleRow

    with tile.TileContext(nc) as tc:
        with (
            tc.tile_pool(name="singles", bufs=1) as singles,
            tc.tile_pool(name="xs", bufs=3) as xpool,
            tc.tile_pool(name="work", bufs=3) as work,
            tc.tile_pool(name="hbuf", bufs=2) as hpool,
            tc.tile_pool(name="psum", bufs=4, space="PSUM") as psum,
        ):
            # PE warm-up: the HAM clock gate holds the PE at 1.2 GHz until it
            # has been busy ~3.4 us.  The PE sits idle anyway while the first
            # DMAs land, so burn that time on dummy matmuls over a zeroed
            # tile — the first real matmuls then run at 2.4 GHz.
            warm = singles.tile([128, TS], F32, tag="warm")
            nc.gpsimd.memset(warm[:], 0.0)
            wps = psum.tile([128, TS], F32, tag="kp")
            for i in range(12):
                nc.tensor.matmul(
                    wps[:], lhsT=warm[:, :128].bitcast(F32R),
                    rhs=warm[:].bitcast(F32R),
                    start=(i == 0), stop=(i == 11),
                )
            # Biases first: tiny but they gate every activation.
            bz_sb = singles.tile([128, NM], F32, tag="bz")
            nc.sync.dma_start(out=bz_sb, in_=bz_d.ap().rearrange("(m p) -> p m", p=128))
            bh_sb = singles.tile([128, NM], F32, tag="bh")
            nc.sync.dma_start(out=bh_sb, in_=bh_d.ap().rearrange("(m p) -> p m", p=128))
            # First strip of x + the m<4 half of the weights, interleaved per
            # k-pair so matmul (s=0, m=0, j=0) unblocks after ~3 transfers.
            xs0 = xpool.tile([128, NK, TS], F8, tag="xs")
            wz_sb = singles.tile([128, NK, H], F8, tag="wz")
            wh_sb = singles.tile([128, NK, H], F8, tag="wh")
            for j in range(NKP):
                ksl = slice(2 * j, 2 * j + 2)
                nc.sync.dma_start(out=xs0[:, ksl, :], in_=xp_d.ap()[:, 0, ksl, :])
                nc.sync.dma_start(out=wz_sb[:, ksl, :H // 2],
                                  in_=wz_d.ap()[:, ksl, :H // 2])
                nc.sync.dma_start(out=wh_sb[:, ksl, :H // 2],
                                  in_=wh_d.ap()[:, ksl, :H // 2])
            for j in range(NKP):
                ksl = slice(2 * j, 2 * j + 2)
                nc.sync.dma_start(out=wz_sb[:, ksl, H // 2:],
                                  in_=wz_d.ap()[:, ksl, H // 2:])
                nc.sync.dma_start(out=wh_sb[:, ksl, H // 2:],
                                  in_=wh_d.ap()[:, ksl, H // 2:])
            # The last 512-wide strip is split in two 256-wide strips (with b
            # on the DVE): the end-of-kernel pipeline drain runs on half-width
            # tiles, halving the post-matmul tail.
            strips = [(s, 0, TS) for s in range(nt - 1)]
            strips += [(nt - 1, 0, TS // 2), (nt - 1, TS // 2, TS // 2)]
            h_prev: list = [None] * NM

            def post_gemm(m, kp, pp, tw, ts_sl, last):
                """Gate math + scan + store for one (strip, m) unit."""
                z = work.tile([128, TS], F32, tag="z")
                nc.scalar.activation(
                    out=z[:, :tw], in_=kp[:, :tw], func=AF.Sigmoid,
                    bias=bz_sb[:, m:m + 1], scale=DESCALE,
                )
                sp = work.tile([128, TS], F32, tag="sp")
                nc.scalar.activation(
                    out=sp[:, :tw], in_=pp[:, :tw], func=AF.Sigmoid,
                    bias=bh_sb[:, m:m + 1], scale=DESCALE,
                )
                rp = work.tile([128, TS], F32, tag="rp")
                nc.scalar.activation(
                    out=rp[:, :tw], in_=pp[:, :tw], func=AF.Relu,
                    bias=bh_sb[:, m:m + 1], scale=DESCALE,
                )
                # a = 1 - z
                a = work.tile([128, TS], F32, tag="a")
                nc.vector.tensor_scalar(
                    out=a[:, :tw], in0=z[:, :tw], scalar1=-1.0, scalar2=1.0,
                    op0=OP.mult, op1=OP.add,
                )
                # g = min(sigmoid(p+bh), 0.5) + relu(p+bh)
                g = work.tile([128, TS], F32, tag="g")
                nc.vector.scalar_tensor_tensor(
                    out=g[:, :tw], in0=sp[:, :tw], scalar=0.5, in1=rp[:, :tw],
                    op0=OP.min, op1=OP.add,
                )
                # b = z * g  (GpSimd: keeps DVE under the PE roofline;
                # DVE on the final strips, where drain latency matters)
                b = work.tile([128, TS], F32, tag="b")
                beng = nc.vector if last else nc.gpsimd
                beng.tensor_tensor(
                    out=b[:, :tw], in0=z[:, :tw], in1=g[:, :tw], op=OP.mult
                )
                # h_t = a_t * h_{t-1} + b_t along the free axis
                h = hpool.tile([128, TS], F32, tag=f"h{m}")
                if h_prev[m] is None:
                    init = 0.5
                else:
                    pt, pw = h_prev[m]
                    init = pt[:, pw - 1:pw]
                nc.vector.tensor_tensor_scan(
                    out=h[:, :tw], data0=a[:, :tw], data1=b[:, :tw],
                    initial=init, op0=OP.mult, op1=OP.add,
                )
                h_prev[m] = (h, tw)
                nc.sync.dma_start(out=hT_d.ap()[m * 128:(m + 1) * 128, ts_sl],
                                  in_=h[:, :tw])

            for s, (sidx, off, tw) in enumerate(strips):
                ts0 = sidx * TS + off
                ts_sl = slice(ts0, ts0 + tw)
                last = ts0 + tw == seq_len
                if sidx == 0:
                    xs = xs0
                elif off == 0:
                    xs = xpool.tile([128, NK, TS], F8, tag="xs")
                    nc.sync.dma_start(out=xs, in_=xp_d.ap()[:, sidx, :, :])
                # (for the second half-strip, reuse the tile loaded at off==0)
                x_sl = slice(off, off + tw)
                for m in range(NM):
                    m_sl = slice(m * 128, (m + 1) * 128)
                    kp = psum.tile([128, TS], F32, tag="kp")
                    pp = psum.tile([128, TS], F32, tag="pp")
                    for j in range(NKP):
                        ksl = slice(2 * j, 2 * j + 2)
                        nc.tensor.matmul(
                            kp[:, :tw],
                            lhsT=wz_sb[:, ksl, m_sl],
                            rhs=xs[:, ksl, x_sl],
                            start=(j == 0),
                            stop=(j == NKP - 1),
                            perf_mode=DR,
                        )
                    for j in range(NKP):
                        ksl = slice(2 * j, 2 * j + 2)
                        nc.tensor.matmul(
                            pp[:, :tw],
                            lhsT=wh_sb[:, ksl, m_sl],
                            rhs=xs[:, ksl, x_sl],
                            start=(j == 0),
                            stop=(j == NKP - 1),
                            perf_mode=DR,
                        )
                    post_gemm(m, kp, pp, tw, ts_sl, last)

    nc.compile()
    return nc


def quantize_pack_x(x_b: np.ndarray, seq_len: int = S) -> np.ndarray:
    """x_b (seq, D) fp32 -> packed [128, nt, NK, TS] fp8 (scaled by SX)."""
    nt = seq_len // TS
    x8 = np.asarray(x_b * SX, dtype=NP_F8)
    return np.ascontiguousarray(
        x8.reshape(nt, TS, NK, 128).transpose(3, 0, 2, 1)
    )


def quantize_pack_w(W: np.ndarray) -> np.ndarray:
    """W (H, D) fp32 -> packed [128, NK, H] fp8 (scaled by SW)."""
    W8 = np.asarray(W * SW, dtype=NP_F8)
    # w8[p, kt, m] = W[m, kt*128+p] * SW
    return np.ascontiguousarray(W8.T.reshape(NK, 128, H).transpose(1, 0, 2))


def make_in_maps(x, Wz, bz, Wh, bh, seq_len: int = S):
    wz8 = quantize_pack_w(np.asarray(Wz, np.float32))
    wh8 = quantize_pack_w(np.asarray(Wh, np.float32))
    bz = np.ascontiguousarray(bz, dtype=np.float32)
    bh = np.ascontiguousarray(bh, dtype=np.float32)
    return [
        {
            "xp8": quantize_pack_x(np.asarray(x[b], np.float32), seq_len),
            "wz8": wz8,
            "wh8": wh8,
            "bz": bz,
            "bh": bh,
        }
        for b in range(x.shape[0])
    ]


def kernel(x, Wz, bz, Wh, bh):
    x = np.ascontiguousarray(x, dtype=np.float32)
    key = "nc"
    if key not in _cache:
        _cache[key] = build_nc()
    nc = _cache[key]

    in_maps = make_in_maps(x, Wz, bz, Wh, bh)
    res = run_bass_kernel_spmd(nc, in_maps, list(range(N_CORES)))
    out = np.empty((B, S, H), np.float32)
    for b in range(N_CORES):
        out[b] = res.results[b]["hT"].T
    return out


# revision 3
# speedup vs baseline: 1.3014x; 1.1861x over previous
"""MinGRU Trainium2 kernel.

Problem: x (8, 4096, 1024) fp32; Wz, Wh (1024, 1024); bz, bh (1024,).
    k = x @ Wz.T + bz ; z = sigmoid(k)
    p = x @ Wh.T + bh ; g = where(p >= 0, p + 0.5, sigmoid(p))
    h_t = (1 - z_t) * h_{t-1} + z_t * g_t   (h_0 = 0.5)
The reference computes this recurrence with a log-space parallel scan; here it
is computed directly in linear space (mathematically identical), using the DVE
TensorTensorScanArith instruction along the free axis.

Sharding: data-parallel over batch, one batch element per NeuronCore (8 cores).

Per-core layout: everything lives transposed, H on partitions, S on the free
axis.  The two GEMMs run in fp8 e4m3 with perf_mode=DoubleRow (2 weights per
PE cell, K=256 per matmul -> half the matmul count of full-rate fp32).  Inputs
are quantized host-side with power-of-two scales (x*16, W*1024); the exact
descale 2^-14 is folded into the ScalarE activation `scale` argument.

Work is chunked in 1024-wide sequence units (PSUM tiles span 2 banks) to
amortize the per-instruction overheads (~352 cycles per ACT, ~200 per DVE op)
and halve the semaphore traffic.  Engine assignment keeps the per-unit chain
g -> b -> scan on the DVE alone (strict-FIFO queues: a cross-engine wait at
the queue head stalls everything behind it):
    ScalarE: z = sigmoid(kp), sp = sigmoid(pp), rp = relu(pp)   (bias fused)
    GpSimd:  a = 1 - z
    DVE:     g = min(sp,.5)+rp ; b = z*g ; h = scan(a, b)
z and a stay fp32 (the scan coefficient); sp/rp/g/b and the stored h are bf16
(the DVE runs 16-bit ops at 2x).  Measured end-to-end rel-err ~1.3e-2 against
the fp32 reference, within the 2e-2 budget.
"""

import os
import sys

import numpy as np

for _p in ("/opt/trn_rl_repo", "/root/.axon_site/_ro/trn_rl_repo"):
    if os.path.isdir(_p) and _p not in sys.path:
        sys.path.insert(0, _p)

import ml_dtypes  # noqa: E402

import concourse.bass as bass  # noqa: E402
import concourse.mybir as mybir  # noqa: E402
import concourse.tile as tile  # noqa: E402
from concourse import bacc  # noqa: E402
from concourse.bass_utils import run_bass_kernel_spmd  # noqa: E402

F32 = mybir.dt.float32
F32R = mybir.dt.float32r
BF16 = mybir.dt.bfloat16
F8 = mybir.dt.float8e4  # TRN e4m3 (bias 8, max +-240) == ml_dtypes.float8_e4m3
NP_F8 = ml_dtypes.float8_e4m3
NP_BF16 = ml_dtypes.bfloat16
N_CORES = 8
B, S, D, H = 8, 4096, 1024, 1024
NK = D // 128  # 8 k-tiles of 128
NKP = NK // 2  # 4 DoubleRow k-pairs
NM = H // 128

# power-of-two quantization scales; descale folded into the activations
SX = 16.0
SW = 1024.0
DESCALE = 1.0 / (SX * SW)

_cache: dict = {}


def build_nc(seq_len: int = S, n_cores: int = N_CORES):
    """Build and compile the per-core Bass module (SPMD, identical program)."""
    tsp = min(1024, seq_len)  # strip width (2 PSUM banks of fp32 at 1024)
    nst = seq_len // tsp
    nc = bacc.Bacc(
        "TRN2", target_bir_lowering=False, debug=False, num_devices=n_cores
    )

    # x packed host-side as [p, strip, ktile, t] so one DMA fetches a strip
    xp_d = nc.dram_tensor("xp8", [128, nst, NK, tsp], F8, kind="ExternalInput")
    # weights packed as [p, ktile, m] (wz8[p, kt, m] = Wz[m, kt*128+p] * SW)
    wz_d = nc.dram_tensor("wz8", [128, NK, H], F8, kind="ExternalInput")
    wh_d = nc.dram_tensor("wh8", [128, NK, H], F8, kind="ExternalInput")
    bz_d = nc.dram_tensor("bz", [H], F32, kind="ExternalInput")
    bh_d = nc.dram_tensor("bh", [H], F32, kind="ExternalInput")
    hT_d = nc.dram_tensor("hT", [H, seq_len], BF16, kind="ExternalOutput")

    AF = mybir.ActivationFunctionType
    OP = mybir.AluOpType
    DR = mybir.MatmulPerfMode.DoubleRow

    with tile.TileContext(nc) as tc:
        with (
            tc.tile_pool(name="singles", bufs=1) as singles,
            tc.tile_pool(name="xs", bufs=3) as xpool,
            tc.tile_pool(name="work", bufs=3) as work,
            tc.tile_pool(name="hbuf", bufs=2) as hpool,
            tc.tile_pool(name="psum", bufs=2, space="PSUM") as psum,
        ):
            # PE warm-up: the HAM clock gate holds the PE at 1.2 GHz until it
            # has been busy ~3.4 us.  The PE sits idle anyway while the first
            # DMAs land, so burn that time on dummy matmuls over a zeroed
            # tile — the first real matmuls then run at 2.4 GHz.
            warm = singles.tile([128, 512], F32, tag="warm")
            nc.gpsimd.memset(warm[:], 0.0)
            wps = psum.tile([128, tsp], F32, tag="kp")
            for i in range(12):
                nc.tensor.matmul(
                    wps[:, :512], lhsT=warm[:, :128].bitcast(F32R),
                    rhs=warm[:].bitcast(F32R),
                    start=(i == 0), stop=(i == 11),
                )
            # Biases first: tiny but they gate every activation.
            bz_sb = singles.tile([128, NM], F32, tag="bz")
            nc.sync.dma_start(out=bz_sb, in_=bz_d.ap().rearrange("(m p) -> p m", p=128))
            bh_sb = singles.tile([128, NM], F32, tag="bh")
            nc.sync.dma_start(out=bh_sb, in_=bh_d.ap().rearrange("(m p) -> p m", p=128))
            # First strip of x + the m<4 half of the weights, interleaved per
            # k-pair so matmul (s=0, m=0, j=0) unblocks after ~3 transfers.
            xs0 = xpool.tile([128, NK, tsp], F8, tag="xs")
            wz_sb = singles.tile([128, NK, H], F8, tag="wz")
            wh_sb = singles.tile([128, NK, H], F8, tag="wh")
            for j in range(NKP):
                ksl = slice(2 * j, 2 * j + 2)
                nc.sync.dma_start(out=xs0[:, ksl, :], in_=xp_d.ap()[:, 0, ksl, :])
                nc.sync.dma_start(out=wz_sb[:, ksl, :H // 2],
                                  in_=wz_d.ap()[:, ksl, :H // 2])
                nc.sync.dma_start(out=wh_sb[:, ksl, :H // 2],
                                  in_=wh_d.ap()[:, ksl, :H // 2])
            for j in range(NKP):
                ksl = slice(2 * j, 2 * j + 2)
                nc.sync.dma_start(out=wz_sb[:, ksl, H // 2:],
                                  in_=wz_d.ap()[:, ksl, H // 2:])
                nc.sync.dma_start(out=wh_sb[:, ksl, H // 2:],
                                  in_=wh_d.ap()[:, ksl, H // 2:])
            # Sequence units: full strips of `tsp`, with the final strip split
            # into halves/quarters so the end-of-kernel pipeline drain runs on
            # narrow tiles.
            units = [(s, 0, tsp) for s in range(nst - 1)]
            units += [(nst - 1, 0, tsp // 2),
                      (nst - 1, tsp // 2, tsp // 4),
                      (nst - 1, 3 * tsp // 4, tsp // 4)]
            h_prev: list = [None] * NM

            def post_gemm(m, kp, pp, tw, ts_sl):
                """Gate math + scan + store for one (unit, m)."""
                z = work.tile([128, tsp], F32, tag="z")
                nc.scalar.activation(
                    out=z[:, :tw], in_=kp[:, :tw], func=AF.Sigmoid,
                    bias=bz_sb[:, m:m + 1], scale=DESCALE,
                )
                sp = work.tile([128, tsp], BF16, tag="sp")
                nc.scalar.activation(
                    out=sp[:, :tw], in_=pp[:, :tw], func=AF.Sigmoid,
                    bias=bh_sb[:, m:m + 1], scale=DESCALE,
                )
                rp = work.tile([128, tsp], BF16, tag="rp")
                nc.scalar.activation(
                    out=rp[:, :tw], in_=pp[:, :tw], func=AF.Relu,
                    bias=bh_sb[:, m:m + 1], scale=DESCALE,
                )
                # a = 1 - z (GpSimd — off the DVE, which owns g/b/scan)
                a = work.tile([128, tsp], F32, tag="a")
                nc.gpsimd.tensor_scalar(
                    out=a[:, :tw], in0=z[:, :tw], scalar1=-1.0, scalar2=1.0,
                    op0=OP.mult, op1=OP.add,
                )
                # g = min(sigmoid(p+bh), 0.5) + relu(p+bh)
                g = work.tile([128, tsp], BF16, tag="g")
                nc.vector.scalar_tensor_tensor(
                    out=g[:, :tw], in0=sp[:, :tw], scalar=0.5, in1=rp[:, :tw],
                    op0=OP.min, op1=OP.add,
                )
                # b = z * g (DVE: keeps the g->b->scan chain on one engine,
                # so the strict-FIFO queue never head-of-line blocks)
                b = work.tile([128, tsp], BF16, tag="b")
                nc.vector.tensor_tensor(
                    out=b[:, :tw], in0=z[:, :tw], in1=g[:, :tw], op=OP.mult
                )
                # h_t = a_t * h_{t-1} + b_t along the free axis
                h = hpool.tile([128, tsp], BF16, tag=f"h{m}")
                if h_prev[m] is None:
                    init = 0.5
                else:
                    pt, pw = h_prev[m]
                    init = pt[:, pw - 1:pw]
                nc.vector.tensor_tensor_scan(
                    out=h[:, :tw], data0=a[:, :tw], data1=b[:, :tw],
                    initial=init, op0=OP.mult, op1=OP.add,
                )
                h_prev[m] = (h, tw)
                nc.sync.dma_start(out=hT_d.ap()[m * 128:(m + 1) * 128, ts_sl],
                                  in_=h[:, :tw])

            for u, (sidx, off, tw) in enumerate(units):
                ts0 = sidx * tsp + off
                ts_sl = slice(ts0, ts0 + tw)
                if sidx == 0:
                    xs = xs0
                elif off == 0:
                    xs = xpool.tile([128, NK, tsp], F8, tag="xs")
                    nc.sync.dma_start(out=xs, in_=xp_d.ap()[:, sidx, :, :])
                # (tail sub-units reuse the strip tile loaded at off==0)
                blocks = [(off + i, min(512, tw - i)) for i in range(0, tw, 512)]
                for m in range(NM):
                    m_sl = slice(m * 128, (m + 1) * 128)
                    kp = psum.tile([128, tsp], F32, tag="kp")
                    pp = psum.tile([128, tsp], F32, tag="pp")
                    for wsb, out_ps in ((wz_sb, kp), (wh_sb, pp)):
                        for j in range(NKP):
                            ksl = slice(2 * j, 2 * j + 2)
                            for bo, bw in blocks:
                                nc.tensor.matmul(
                                    out_ps[:, bo - off:bo - off + bw],
                                    lhsT=wsb[:, ksl, m_sl],
                                    rhs=xs[:, ksl, bo:bo + bw],
                                    start=(j == 0),
                                    stop=(j == NKP - 1),
                                    perf_mode=DR,
                                )
                    post_gemm(m, kp, pp, tw, ts_sl)

    nc.compile()
    return nc


def quantize_pack_x(x_b: np.ndarray, seq_len: int = S) -> np.ndarray:
    """x_b (seq, D) fp32 -> packed [128, nst, NK, tsp] fp8 (scaled by SX)."""
    tsp = min(1024, seq_len)
    nst = seq_len // tsp
    x8 = np.asarray(x_b * SX, dtype=NP_F8)
    return np.ascontiguousarray(
        x8.reshape(nst, tsp, NK, 128).transpose(3, 0, 2, 1)
    )


def quantize_pack_w(W: np.ndarray) -> np.ndarray:
    """W (H, D) fp32 -> packed [128, NK, H] fp8 (scaled by SW)."""
    W8 = np.asarray(W * SW, dtype=NP_F8)
    # w8[p, kt, m] = W[m, kt*128+p] * SW
    return np.ascontiguousarray(W8.T.reshape(NK, 128, H).transpose(1, 0, 2))


def make_in_maps(x, Wz, bz, Wh, bh, seq_len: int = S):
    wz8 = quantize_pack_w(np.asarray(Wz, np.float32))
    wh8 = quantize_pack_w(np.asarray(Wh, np.float32))
    bz = np.ascontiguousarray(bz, dtype=np.float32)
    bh = np.ascontiguousarray(bh, dtype=np.float32)
    return [
        {
            "xp8": quantize_pack_x(np.asarray(x[b], np.float32), seq_len),
            "wz8": wz8,
            "wh8": wh8,
            "bz": bz,
            "bh": bh,
        }
        for b in range(x.shape[0])
    ]


def kernel(x, Wz, bz, Wh, bh):
    x = np.ascontiguousarray(x, dtype=np.float32)
    key = "nc"
    if key not in _cache:
        _cache[key] = build_nc()
    nc = _cache[key]

    in_maps = make_in_maps(x, Wz, bz, Wh, bh)
    res = run_bass_kernel_spmd(nc, in_maps, list(range(N_CORES)))
    out = np.empty((B, S, H), np.float32)
    for b in range(N_CORES):
        out[b] = res.results[b]["hT"].astype(np.float32).T
    return out
